# revision 16
# baseline (speedup 1.0000x reference)
"""MechanismAttention Trainium2 Bass kernel (8-core SPMD).

Sharding: core c -> (batch b = c//4, head group g = c%4, 4 heads each).
Each core computes q/k/v projections for its 4 heads, the mechanism gate
(replicated within its batch group), gated attention, and a partial output
projection (rows of Wo for its heads). Host sums the 4 partials per batch
and adds bo.

Layout tricks:
  - x is passed host-transposed as xT [1152, 2048]: rows 0-1023 = x[b].T,
    row 1024 = ones (folds projection biases via an extra contraction row),
    rows 1025-1151 zero padding to a multiple of 128.
  - scores are computed transposed st[k, q] so the mechanism gate (per key
    token = per partition) fuses into the Exp activation's per-partition
    scale: st_exp = exp(q.k * (1+sigma_k)/8). No max-subtraction needed
    (|scores| <~ 16 so exp stays in fp32 range).
  - v carries an appended ones-column, so the attention matmul emits the
    softmax denominator as column 64 of the context tile; normalization is
    then a per-partition reciprocal+scale in token-major layout.
"""

import os
import sys

import numpy as np

sys.path.insert(0, "/opt/trn_rl_repo")

P = 128
T = 2048
NT = T // P  # 16 token tiles
DMODEL = 1024
KK = 9  # contraction tiles over padded dmodel (1152 = 9*128)
DH = 64
NH = 4  # heads per core
DMECH = 512
NCORES = 8

TRACE = bool(int(os.environ.get("KERNEL_TRACE", "0")))
LAST_RESULTS = None

_COMPILED = None


def _enable_ldw_opt():
    """Walrus ships with --enable-ldw-opt=false; turning it on merges
    back-to-back weight loads of the same tile (our ctx/scores matmuls
    reuse the stationary operand 2-4x). Results are verified by the
    caller's rel-err check."""
    from concourse import bass_utils

    if getattr(bass_utils, "_ldw_patched", False):
        return
    orig = bass_utils.run_command

    def patched(argv, **kwargs):
        argv = [
            a.replace("--enable-ldw-opt=false", "--enable-ldw-opt=true")
            if isinstance(a, str)
            else a
            for a in argv
        ]
        return orig(argv, **kwargs)

    bass_utils.run_command = patched
    bass_utils._ldw_patched = True


def _build_program():
    import concourse.bass as bass
    import concourse.tile as tile
    from concourse import bacc, mybir
    from concourse.masks import make_identity

    if os.environ.get("KERNEL_LDW_OPT", "0") == "1":
        _enable_ldw_opt()

    f32 = mybir.dt.float32
    f32r = mybir.dt.float32r
    bf16 = mybir.dt.bfloat16
    AF = mybir.ActivationFunctionType

    nc = bacc.Bacc(
        "TRN2",
        target_bir_lowering=False,
        debug=False,
        num_devices=NCORES,
    )

    xT = nc.dram_tensor("xT", [P, KK, T], f32r, kind="ExternalInput").ap()
    wq = nc.dram_tensor("wq", [P, KK, 256], f32r, kind="ExternalInput").ap()
    wk = nc.dram_tensor("wk", [P, KK, 256], f32r, kind="ExternalInput").ap()
    wv = nc.dram_tensor("wv", [P, KK, 256], f32r, kind="ExternalInput").ap()
    wo = nc.dram_tensor("wo", [P, 2, DMODEL], f32r, kind="ExternalInput").ap()
    wm1 = nc.dram_tensor("wm1", [P, KK, 128], f32r, kind="ExternalInput").ap()
    wm2 = nc.dram_tensor("wm2", [P, 1], f32r, kind="ExternalInput").ap()
    bm2b = nc.dram_tensor("bm2b", [P, 1], f32, kind="ExternalInput").ap()
    outp = nc.dram_tensor("outp", [P, NT, DMODEL], f32, kind="ExternalOutput").ap()
    souts = nc.dram_tensor("souts", [P, NT], f32, kind="ExternalOutput").ap()

    with tile.TileContext(nc) as tc:
        with tc.tile_pool(name="persist", bufs=1) as persist:
            identity = persist.tile([P, P], f32)
            make_identity(nc, identity)

            # persistent attention operands
            qT = persist.tile([P, 2, T], f32r)  # [d-in-pack, pack, token]
            kT = persist.tile([P, 2, T], f32r)
            vv = persist.tile([P, NT, NH * 66], f32r)  # 4x(64 v, 1 one, 1 pad): f32r needs even N
            scl = persist.tile([P, NT], f32)  # (1+sigmoid)/8, token-partition
            ssb = persist.tile([P, NT], f32)  # sigmoid, token-partition
            ctxT = persist.tile([P, 2, T], f32r)  # ctx^T rows (h,d), token cols

            # ---------------- Phase 1: projections + mechanism ----------------
            with tc.tile_pool(name="p1w", bufs=1) as p1w, \
                 tc.tile_pool(name="p1psum", bufs=2, space="PSUM") as pp1, \
                 tc.tile_pool(name="p1spsum", bufs=2, space="PSUM") as pps, \
                 tc.tile_pool(name="p1tmp", bufs=2) as p1tmp, \
                 tc.tile_pool(name="p1dram", bufs=1, space="DRAM") as p1dram:
                xs_k = []
                for kk in range(KK):
                    t = p1w.tile([P, T], f32r, tag=f"xs{kk}")
                    nc.sync.dma_start(t, xT[:, kk, :])
                    xs_k.append(t)
                wqs = p1w.tile([P, KK, 256], f32r)
                nc.sync.dma_start(wqs, wq)
                wks = p1w.tile([P, KK, 256], f32r)
                nc.sync.dma_start(wks, wk)
                wvs = p1w.tile([P, KK, 256], f32r)
                nc.sync.dma_start(wvs, wv)
                wm1s = p1w.tile([P, KK, 128], f32r)
                nc.sync.dma_start(wm1s, wm1)
                wm2s = p1w.tile([P, 1], f32r)
                nc.sync.dma_start(wm2s, wm2)
                bm2s = p1w.tile([P, 1], f32)
                nc.sync.dma_start(bm2s, bm2b)

                # qT / kT: out[m = 128 (2 heads x 64), n = token chunk]
                for w_sb, dst in ((wqs, qT), (wks, kT)):
                    for pack in range(2):
                        for ch in range(4):
                            ps = pp1.tile([P, 512], f32, tag="projps")
                            for kk in range(KK):
                                nc.tensor.matmul(
                                    ps,
                                    w_sb[:, kk, pack * 128 : (pack + 1) * 128],
                                    xs_k[kk][:, ch * 512 : (ch + 1) * 512],
                                    start=(kk == 0),
                                    stop=(kk == KK - 1),
                                )
                            nc.scalar.activation(
                                dst[:, pack, ch * 512 : (ch + 1) * 512], ps, AF.Copy
                            )

                # v natural [token, 4 heads x 64] + ones columns.
                # memset can't produce float32r (ISA check); route the ones
                # through an ACT copy, which rounds to f32r.
                ones_f = p1tmp.tile([P, NT], f32, tag="ones_f")
                nc.vector.memset(ones_f, 1.0)
                for h in range(NH):
                    for cc in (64, 65):
                        nc.scalar.activation(
                            vv[:, :, h * 66 + cc : h * 66 + cc + 1],
                            ones_f[:, :, None],
                            AF.Copy,
                        )
                for tt in range(NT):
                    ps = pp1.tile([P, 256], f32, tag="vps")
                    for kk in range(KK):
                        nc.tensor.matmul(
                            ps,
                            xs_k[kk][:, tt * 128 : (tt + 1) * 128],
                            wvs[:, kk, :],
                            start=(kk == 0),
                            stop=(kk == KK - 1),
                        )
                    for h in range(NH):
                        nc.vector.tensor_copy(
                            vv[:, tt, h * 66 : h * 66 + 64],
                            ps[:, h * 64 : (h + 1) * 64],
                        )

                # mechanism: each core computes a 128-wide slice of the
                # gate MLP hidden dim (tensor parallel within its batch
                # group), then AllReduce-adds the [128, NT] strength partials
                # across the 4 cores of the group. PSUM accumulation groups
                # close immediately (they must not interleave); N=1 matmuls
                # run as plain fp32 (fp32r ISA rejects odd N).
                hT_t = p1tmp.tile([P, T], f32r, tag="hT")
                for ch in range(4):
                    ps = pp1.tile([P, 512], f32, tag="projps")
                    for kk in range(KK):
                        nc.tensor.matmul(
                            ps,
                            wm1s[:, kk, :],
                            xs_k[kk][:, ch * 512 : (ch + 1) * 512],
                            start=(kk == 0),
                            stop=(kk == KK - 1),
                        )
                    nc.scalar.activation(
                        hT_t[:, ch * 512 : (ch + 1) * 512], ps, AF.Gelu
                    )
                s_ps = pps.tile([P, NT], f32, tag="s_ps")
                for tt in range(NT):
                    nc.tensor.matmul(
                        s_ps[:, tt : tt + 1],
                        hT_t[:, tt * 128 : (tt + 1) * 128].bitcast(f32),
                        wm2s.bitcast(f32),
                        start=True,
                        stop=True,
                    )
                s_part = p1tmp.tile([P, NT], f32, tag="s_part")
                nc.vector.tensor_copy(s_part, s_ps)
                s_bounce_in = p1dram.tile([P, NT], f32)
                s_bounce_out = p1dram.tile([P, NT], f32)
                nc.gpsimd.dma_start(s_bounce_in, s_part)
                nc.gpsimd.collective_compute(
                    "AllReduce",
                    mybir.AluOpType.add,
                    replica_groups=[[0, 1, 2, 3], [4, 5, 6, 7]],
                    ins=[s_bounce_in.opt()],
                    outs=[s_bounce_out.opt()],
                )
                s_acc = p1tmp.tile([P, NT], f32, tag="s_acc")
                nc.sync.dma_start(s_acc, s_bounce_out)
                nc.scalar.activation(ssb, s_acc, AF.Sigmoid, bias=bm2s)
                nc.scalar.activation(scl, ssb, AF.Copy, bias=0.125, scale=0.125)
                nc.sync.dma_start(souts, ssb)

            # ---------------- Phase 2: gated attention ----------------
            # scores are built transposed st[k, q] per key-tile; exp fuses the
            # mechanism gate as a per-partition scale. The context matmul keeps
            # v' stationary (lhsT [128, 66]) and streams st (N=512, full-rate
            # fp32r), accumulating ctxT' [66, 2048] per head across key tiles
            # in four single-bank psum tiles (groups may interleave across
            # banks, never within one). Normalization transposes each token
            # tile to token-major, divides by the ones-column sum, and
            # transposes back into ctxT for the output projection.
            with tc.tile_pool(name="stp", bufs=2) as stp, \
                 tc.tile_pool(name="sps", bufs=2, space="PSUM") as sps, \
                 tc.tile_pool(name="cps", bufs=1, space="PSUM") as cps, \
                 tc.tile_pool(name="ctmp", bufs=4) as ctmp:
                for h in range(NH):
                    pk = h // 2
                    po = (h % 2) * 64
                    ctxp = [
                        cps.tile([66, 512], f32, tag=f"cx{c}", name=f"cx{c}") for c in range(4)
                    ]
                    for kt in range(NT):
                        stt = stp.tile([P, T], f32r, tag="st")
                        for hf in range(2):
                            ps = sps.tile([P, 1024], f32, tag="stps")
                            for q2 in range(2):
                                nc.tensor.matmul(
                                    ps[:, q2 * 512 : (q2 + 1) * 512],
                                    kT[po : po + 64, pk, kt * 128 : (kt + 1) * 128],
                                    qT[
                                        po : po + 64,
                                        pk,
                                        hf * 1024 + q2 * 512 : hf * 1024 + (q2 + 1) * 512,
                                    ],
                                    start=True,
                                    stop=True,
                                )
                            nc.scalar.activation(
                                stt[:, hf * 1024 : (hf + 1) * 1024],
                                ps,
                                AF.Exp,
                                scale=scl[:, kt : kt + 1],
                            )
                        for c in range(4):
                            nc.tensor.matmul(
                                ctxp[c],
                                vv[:, kt, h * 66 : (h + 1) * 66],
                                stt[:, c * 512 : (c + 1) * 512],
                                start=(kt == 0),
                                stop=(kt == NT - 1),
                            )
                    # normalize + transpose into ctxT, batched 4 token
                    # tiles per engine round-trip to keep sem latency off the
                    # critical path
                    ctn = ctmp.tile([66, T], f32, tag="ctn")
                    for c in range(4):
                        nc.vector.tensor_copy(
                            ctn[:, c * 512 : (c + 1) * 512], ctxp[c]
                        )
                    for g in range(4):
                        tp1 = cps.tile([P, 4, 66], f32, tag=f"cx{g % 2}", name="tp1")
                        for j in range(4):
                            tt = g * 4 + j
                            nc.tensor.transpose(
                                tp1[:, j, :],
                                ctn[:, tt * 128 : (tt + 1) * 128],
                                identity[:66, :66],
                            )
                        rec = ctmp.tile([P, 4, 1], f32, tag="rec")
                        nc.vector.reciprocal(rec, tp1[:, :, 64:65])
                        cn = ctmp.tile([P, 4, DH], f32, tag="cn")
                        nc.vector.tensor_tensor(
                            cn,
                            tp1[:, :, 0:DH],
                            rec.to_broadcast((P, 4, DH)),
                            mybir.AluOpType.mult,
                        )
                        tp2 = cps.tile([DH, 4, P], f32, tag=f"cx{2 + g % 2}", name="tp2")
                        for j in range(4):
                            nc.tensor.transpose(tp2[:, j, :], cn[:, j, :], identity)
                        nc.vector.tensor_copy(
                            ctxT[po : po + 64, pk, g * 512 : (g + 1) * 512], tp2
                        )

            # ---------------- Phase 3: output projection (partial) ----------------
            with tc.tile_pool(name="p3w", bufs=1) as p3w, \
                 tc.tile_pool(name="p3psum", bufs=2, space="PSUM") as pp3, \
                 tc.tile_pool(name="p3out", bufs=3) as p3o:
                wos = p3w.tile([P, 2, DMODEL], f32r)
                nc.sync.dma_start(wos, wo)
                for tt in range(NT):
                    ob = p3o.tile([P, DMODEL], f32, tag="ob")
                    for hf in range(2):
                        ps = pp3.tile([P, 512], f32, tag="ops")
                        for kk2 in range(2):
                            nc.tensor.matmul(
                                ps,
                                ctxT[:, kk2, tt * 128 : (tt + 1) * 128],
                                wos[:, kk2, hf * 512 : (hf + 1) * 512],
                                start=(kk2 == 0),
                                stop=(kk2 == 1),
                            )
                        nc.vector.tensor_copy(ob[:, hf * 512 : (hf + 1) * 512], ps)
                    nc.sync.dma_start(outp[:, tt, :], ob)

    nc.compile()
    return nc


def _prep_core_inputs(c, x, Wq, bq, Wk, bk, Wv, bv, Wm1, bm1, Wm2, Wo):
    b, g = c // 4, c % 4
    cs = slice(256 * g, 256 * (g + 1))
    f32 = np.float32

    def ptile(a, last):
        # [1152 or 512 or 256 rows, cols] -> [P, rows//P, cols]
        n = a.shape[0] // P
        return np.ascontiguousarray(
            a.reshape(n, P, last).transpose(1, 0, 2)
        )

    xpad = np.zeros((KK * P, T), f32)
    xpad[:DMODEL] = x[b].T
    xpad[DMODEL] = 1.0

    def wpad(W, bias, col_slice):
        wp = np.zeros((KK * P, 256), f32)
        wp[:DMODEL] = W[:, col_slice]
        wp[DMODEL] = bias[col_slice]
        return wp

    ms = slice(128 * g, 128 * (g + 1))
    wm1p = np.zeros((KK * P, 128), f32)
    wm1p[:DMODEL] = Wm1[:, ms]
    wm1p[DMODEL] = bm1[ms]

    return {
        "xT": ptile(xpad, T),
        "wq": ptile(wpad(Wq, bq, cs), 256),
        "wk": ptile(wpad(Wk, bk, cs), 256),
        "wv": ptile(wpad(Wv, bv, cs), 256),
        "wo": ptile(np.ascontiguousarray(Wo[cs], f32), DMODEL),
        "wm1": ptile(wm1p, 128),
        "wm2": np.ascontiguousarray(Wm2[ms], f32).reshape(P, 1),
        "bm2b": None,  # filled by caller
    }


def kernel(x, Wq, bq, Wk, bk, Wv, bv, Wo, bo, Wm1, bm1, Wm2, bm2):
    global _COMPILED, LAST_RESULTS
    from concourse import bass_utils

    x = np.asarray(x, np.float32)
    B = x.shape[0]
    assert x.shape == (2, T, DMODEL)

    if _COMPILED is None:
        _COMPILED = _build_program()
    nc = _COMPILED

    bm2bc = np.full((P, 1), np.float32(np.asarray(bm2).reshape(-1)[0]))
    in_maps = []
    for c in range(NCORES):
        m = _prep_core_inputs(
            c,
            x,
            np.asarray(Wq, np.float32), np.asarray(bq, np.float32),
            np.asarray(Wk, np.float32), np.asarray(bk, np.float32),
            np.asarray(Wv, np.float32), np.asarray(bv, np.float32),
            np.asarray(Wm1, np.float32), np.asarray(bm1, np.float32),
            np.asarray(Wm2, np.float32), np.asarray(Wo, np.float32),
        )
        m["bm2b"] = bm2bc
        in_maps.append(m)

    kw = {}
    if TRACE:
        kw = dict(trace=True, trace_cores=list(range(NCORES)))
    res = bass_utils.run_bass_kernel_spmd(
        nc, in_maps, core_ids=list(range(NCORES)), **kw
    )
    LAST_RESULTS = res

    bo = np.asarray(bo, np.float32)
    output = np.zeros((B, T, DMODEL), np.float32)
    strengths = np.zeros((B, T), np.float32)
    for b in range(B):
        acc = np.zeros((T, DMODEL), np.float32)
        for g in range(4):
            op = res.results[4 * b + g]["outp"]  # [P, NT, DMODEL]
            acc += op.transpose(1, 0, 2).reshape(T, DMODEL)
        output[b] = acc + bo
        strengths[b] = res.results[4 * b]["souts"].transpose(1, 0).reshape(T)
    return output, strengths


# revision 17
# speedup vs baseline: 1.1470x; 1.1470x over previous
"""MechanismAttention Trainium2 Bass kernel (8-core SPMD).

Sharding: core c -> (batch b = c//4, head group g = c%4, 4 heads each).
Each core computes q/k/v projections for its 4 heads, the mechanism gate
(replicated within its batch group), gated attention, and a partial output
projection (rows of Wo for its heads). Host sums the 4 partials per batch
and adds bo.

Layout tricks:
  - x is passed host-transposed as xT [1152, 2048]: rows 0-1023 = x[b].T,
    row 1024 = ones (folds projection biases via an extra contraction row),
    rows 1025-1151 zero padding to a multiple of 128.
  - scores are computed transposed st[k, q] so the mechanism gate (per key
    token = per partition) fuses into the Exp activation's per-partition
    scale: st_exp = exp(q.k * (1+sigma_k)/8). No max-subtraction needed
    (|scores| <~ 16 so exp stays in fp32 range).
  - v carries an appended ones-column, so the attention matmul emits the
    softmax denominator as column 64 of the context tile; normalization is
    then a per-partition reciprocal+scale in token-major layout.
"""

import os
import sys

import numpy as np

sys.path.insert(0, "/opt/trn_rl_repo")

P = 128
T = 2048
NT = T // P  # 16 token tiles
DMODEL = 1024
KK = 9  # contraction tiles over padded dmodel (1152 = 9*128)
DH = 64
NH = 4  # heads per core
DMECH = 512
NCORES = 8

TRACE = bool(int(os.environ.get("KERNEL_TRACE", "0")))
LAST_RESULTS = None

_COMPILED = None


def _enable_ldw_opt():
    """Walrus ships with --enable-ldw-opt=false; turning it on merges
    back-to-back weight loads of the same tile (our ctx/scores matmuls
    reuse the stationary operand 2-4x). Results are verified by the
    caller's rel-err check."""
    from concourse import bass_utils

    if getattr(bass_utils, "_ldw_patched", False):
        return
    orig = bass_utils.run_command

    def patched(argv, **kwargs):
        argv = [
            a.replace("--enable-ldw-opt=false", "--enable-ldw-opt=true")
            if isinstance(a, str)
            else a
            for a in argv
        ]
        return orig(argv, **kwargs)

    bass_utils.run_command = patched
    bass_utils._ldw_patched = True


def _build_program():
    import concourse.bass as bass
    import concourse.tile as tile
    from concourse import bacc, mybir
    from concourse.masks import make_identity

    if os.environ.get("KERNEL_LDW_OPT", "0") == "1":
        _enable_ldw_opt()

    f32 = mybir.dt.float32
    f32r = mybir.dt.float32r
    bf16 = mybir.dt.bfloat16
    AF = mybir.ActivationFunctionType

    nc = bacc.Bacc(
        "TRN2",
        target_bir_lowering=False,
        debug=False,
        num_devices=NCORES,
    )

    xT = nc.dram_tensor("xT", [P, KK, T], f32r, kind="ExternalInput").ap()
    wq = nc.dram_tensor("wq", [P, KK, 256], f32r, kind="ExternalInput").ap()
    wk = nc.dram_tensor("wk", [P, KK, 256], f32r, kind="ExternalInput").ap()
    wv = nc.dram_tensor("wv", [P, KK, 256], f32r, kind="ExternalInput").ap()
    wo = nc.dram_tensor("wo", [P, 2, DMODEL], f32r, kind="ExternalInput").ap()
    wm1 = nc.dram_tensor("wm1", [P, KK, 128], f32r, kind="ExternalInput").ap()
    wm2 = nc.dram_tensor("wm2", [P, 1], f32r, kind="ExternalInput").ap()
    bm2b = nc.dram_tensor("bm2b", [P, 1], f32, kind="ExternalInput").ap()
    outp = nc.dram_tensor("outp", [P, NT, DMODEL], f32, kind="ExternalOutput").ap()
    souts = nc.dram_tensor("souts", [P, NT], f32, kind="ExternalOutput").ap()

    with tile.TileContext(nc) as tc:
        with tc.tile_pool(name="persist", bufs=1) as persist:
            identity = persist.tile([P, P], f32)
            make_identity(nc, identity)

            # persistent attention operands
            qT = persist.tile([P, 2, T], f32r)  # [d-in-pack, pack, token]
            kT = persist.tile([P, 2, T], f32r)
            vv = persist.tile([P, NT, NH * 66], f32r)  # 4x(64 v, 1 one, 1 pad): f32r needs even N
            scl = persist.tile([P, NT], f32)  # (1+sigmoid)/8, token-partition
            ssb = persist.tile([P, NT], f32)  # sigmoid, token-partition
            ctxT = persist.tile([P, 2, T], f32r)  # ctx^T rows (h,d), token cols

            # ---------------- Phase 1: projections + mechanism ----------------
            with tc.tile_pool(name="p1w", bufs=1) as p1w, \
                 tc.tile_pool(name="p1psum", bufs=2, space="PSUM") as pp1, \
                 tc.tile_pool(name="p1spsum", bufs=2, space="PSUM") as pps, \
                 tc.tile_pool(name="p1tmp", bufs=2) as p1tmp, \
                 tc.tile_pool(name="p1dram", bufs=1, space="DRAM") as p1dram:
                xs_k = []
                for kk in range(KK):
                    t = p1w.tile([P, T], f32r, tag=f"xs{kk}")
                    nc.sync.dma_start(t, xT[:, kk, :])
                    xs_k.append(t)
                wm1s = p1w.tile([P, KK, 128], f32r)
                nc.sync.dma_start(wm1s, wm1)
                wm2s = p1w.tile([P, 1], f32r)
                nc.sync.dma_start(wm2s, wm2)
                bm2s = p1w.tile([P, 1], f32)
                nc.sync.dma_start(bm2s, bm2b)
                wqs = p1w.tile([P, KK, 256], f32r)
                nc.sync.dma_start(wqs, wq)
                wks = p1w.tile([P, KK, 256], f32r)
                nc.sync.dma_start(wks, wk)
                wvs = p1w.tile([P, KK, 256], f32r)
                nc.sync.dma_start(wvs, wv)

                # mechanism: each core computes a 128-wide slice of the
                # gate MLP hidden dim (tensor parallel within its batch
                # group), then AllReduce-adds the [128, NT] strength partials
                # across the 4 cores of the group. PSUM accumulation groups
                # close immediately (they must not interleave); N=1 matmuls
                # run as plain fp32 (fp32r ISA rejects odd N).
                hT_t = p1tmp.tile([P, T], f32r, tag="hT")
                for ch in range(4):
                    ps = pp1.tile([P, 512], f32, tag="projps")
                    for kk in range(KK):
                        nc.tensor.matmul(
                            ps,
                            wm1s[:, kk, :],
                            xs_k[kk][:, ch * 512 : (ch + 1) * 512],
                            start=(kk == 0),
                            stop=(kk == KK - 1),
                        )
                    nc.scalar.activation(
                        hT_t[:, ch * 512 : (ch + 1) * 512], ps, AF.Gelu
                    )
                s_ps = pps.tile([P, NT], f32, tag="s_ps")
                for tt in range(NT):
                    nc.tensor.matmul(
                        s_ps[:, tt : tt + 1],
                        hT_t[:, tt * 128 : (tt + 1) * 128].bitcast(f32),
                        wm2s.bitcast(f32),
                        start=True,
                        stop=True,
                    )
                s_part = p1tmp.tile([P, NT], f32, tag="s_part")
                nc.vector.tensor_copy(s_part, s_ps)
                s_bounce_in = p1dram.tile([P, NT], f32)
                s_bounce_out = p1dram.tile([P, NT], f32)
                nc.gpsimd.dma_start(s_bounce_in, s_part)
                nc.gpsimd.collective_compute(
                    "AllReduce",
                    mybir.AluOpType.add,
                    replica_groups=[[0, 1, 2, 3], [4, 5, 6, 7]],
                    ins=[s_bounce_in.opt()],
                    outs=[s_bounce_out.opt()],
                )
                s_acc = p1tmp.tile([P, NT], f32, tag="s_acc")
                nc.sync.dma_start(s_acc, s_bounce_out)
                nc.scalar.activation(ssb, s_acc, AF.Sigmoid, bias=bm2s)
                nc.scalar.activation(scl, ssb, AF.Copy, bias=0.125, scale=0.125)
                nc.sync.dma_start(souts, ssb)

                # qT / kT: out[m = 128 (2 heads x 64), n = token chunk]
                for w_sb, dst in ((wqs, qT), (wks, kT)):
                    for pack in range(2):
                        for ch in range(4):
                            ps = pp1.tile([P, 512], f32, tag="projps")
                            for kk in range(KK):
                                nc.tensor.matmul(
                                    ps,
                                    w_sb[:, kk, pack * 128 : (pack + 1) * 128],
                                    xs_k[kk][:, ch * 512 : (ch + 1) * 512],
                                    start=(kk == 0),
                                    stop=(kk == KK - 1),
                                )
                            nc.scalar.activation(
                                dst[:, pack, ch * 512 : (ch + 1) * 512], ps, AF.Copy
                            )

                # v natural [token, 4 heads x 64] + ones columns.
                # memset can't produce float32r (ISA check); route the ones
                # through an ACT copy, which rounds to f32r.
                ones_f = p1tmp.tile([P, NT], f32, tag="ones_f")
                nc.vector.memset(ones_f, 1.0)
                for h in range(NH):
                    for cc in (64, 65):
                        nc.scalar.activation(
                            vv[:, :, h * 66 + cc : h * 66 + cc + 1],
                            ones_f[:, :, None],
                            AF.Copy,
                        )
                for tt in range(NT):
                    ps = pp1.tile([P, 256], f32, tag="vps")
                    for kk in range(KK):
                        nc.tensor.matmul(
                            ps,
                            xs_k[kk][:, tt * 128 : (tt + 1) * 128],
                            wvs[:, kk, :],
                            start=(kk == 0),
                            stop=(kk == KK - 1),
                        )
                    for h in range(NH):
                        nc.vector.tensor_copy(
                            vv[:, tt, h * 66 : h * 66 + 64],
                            ps[:, h * 64 : (h + 1) * 64],
                        )


            # ---------------- Phase 2: gated attention ----------------
            # scores are built transposed st[k, q] per key-tile; exp fuses the
            # mechanism gate as a per-partition scale. The context matmul keeps
            # v' stationary (lhsT [128, 66]) and streams st (N=512, full-rate
            # fp32r), accumulating ctxT' [66, 2048] per head across key tiles
            # in four single-bank psum tiles (groups may interleave across
            # banks, never within one). Normalization transposes each token
            # tile to token-major, divides by the ones-column sum, and
            # transposes back into ctxT for the output projection.
            with tc.tile_pool(name="stp", bufs=2) as stp, \
                 tc.tile_pool(name="sps", bufs=2, space="PSUM") as sps, \
                 tc.tile_pool(name="cps", bufs=1, space="PSUM") as cps, \
                 tc.tile_pool(name="ctmp", bufs=4) as ctmp:
                for h in range(NH):
                    pk = h // 2
                    po = (h % 2) * 64
                    ctxp = [
                        cps.tile([66, 512], f32, tag=f"cx{c}", name=f"cx{c}") for c in range(4)
                    ]
                    for kt in range(NT):
                        stt = stp.tile([P, T], f32r, tag="st")
                        for hf in range(2):
                            ps = sps.tile([P, 1024], f32, tag="stps")
                            for q2 in range(2):
                                nc.tensor.matmul(
                                    ps[:, q2 * 512 : (q2 + 1) * 512],
                                    kT[po : po + 64, pk, kt * 128 : (kt + 1) * 128],
                                    qT[
                                        po : po + 64,
                                        pk,
                                        hf * 1024 + q2 * 512 : hf * 1024 + (q2 + 1) * 512,
                                    ],
                                    start=True,
                                    stop=True,
                                )
                            nc.scalar.activation(
                                stt[:, hf * 1024 : (hf + 1) * 1024],
                                ps,
                                AF.Exp,
                                scale=scl[:, kt : kt + 1],
                            )
                        for c in range(4):
                            nc.tensor.matmul(
                                ctxp[c],
                                vv[:, kt, h * 66 : (h + 1) * 66],
                                stt[:, c * 512 : (c + 1) * 512],
                                start=(kt == 0),
                                stop=(kt == NT - 1),
                            )
                    # normalize + transpose into ctxT, batched 4 token
                    # tiles per engine round-trip to keep sem latency off the
                    # critical path
                    ctn = ctmp.tile([66, T], f32, tag="ctn")
                    for c in range(4):
                        nc.vector.tensor_copy(
                            ctn[:, c * 512 : (c + 1) * 512], ctxp[c]
                        )
                    for g in range(4):
                        tp1 = cps.tile([P, 4, 66], f32, tag=f"cx{g % 2}", name="tp1")
                        for j in range(4):
                            tt = g * 4 + j
                            nc.tensor.transpose(
                                tp1[:, j, :],
                                ctn[:, tt * 128 : (tt + 1) * 128],
                                identity[:66, :66],
                            )
                        rec = ctmp.tile([P, 4, 1], f32, tag="rec")
                        nc.vector.reciprocal(rec, tp1[:, :, 64:65])
                        cn = ctmp.tile([P, 4, DH], f32, tag="cn")
                        nc.vector.tensor_tensor(
                            cn,
                            tp1[:, :, 0:DH],
                            rec.to_broadcast((P, 4, DH)),
                            mybir.AluOpType.mult,
                        )
                        tp2 = cps.tile([DH, 4, P], f32, tag=f"cx{2 + g % 2}", name="tp2")
                        for j in range(4):
                            nc.tensor.transpose(tp2[:, j, :], cn[:, j, :], identity)
                        nc.vector.tensor_copy(
                            ctxT[po : po + 64, pk, g * 512 : (g + 1) * 512], tp2
                        )

            # ---------------- Phase 3: output projection (partial) ----------------
            with tc.tile_pool(name="p3w", bufs=1) as p3w, \
                 tc.tile_pool(name="p3psum", bufs=2, space="PSUM") as pp3, \
                 tc.tile_pool(name="p3out", bufs=3) as p3o:
                wos = p3w.tile([P, 2, DMODEL], f32r)
                nc.sync.dma_start(wos, wo)
                for tt in range(NT):
                    ob = p3o.tile([P, DMODEL], f32, tag="ob")
                    for hf in range(2):
                        ps = pp3.tile([P, 512], f32, tag="ops")
                        for kk2 in range(2):
                            nc.tensor.matmul(
                                ps,
                                ctxT[:, kk2, tt * 128 : (tt + 1) * 128],
                                wos[:, kk2, hf * 512 : (hf + 1) * 512],
                                start=(kk2 == 0),
                                stop=(kk2 == 1),
                            )
                        nc.vector.tensor_copy(ob[:, hf * 512 : (hf + 1) * 512], ps)
                    nc.sync.dma_start(outp[:, tt, :], ob)

    nc.compile()
    return nc


def _prep_core_inputs(c, x, Wq, bq, Wk, bk, Wv, bv, Wm1, bm1, Wm2, Wo):
    b, g = c // 4, c % 4
    cs = slice(256 * g, 256 * (g + 1))
    f32 = np.float32

    def ptile(a, last):
        # [1152 or 512 or 256 rows, cols] -> [P, rows//P, cols]
        n = a.shape[0] // P
        return np.ascontiguousarray(
            a.reshape(n, P, last).transpose(1, 0, 2)
        )

    xpad = np.zeros((KK * P, T), f32)
    xpad[:DMODEL] = x[b].T
    xpad[DMODEL] = 1.0

    def wpad(W, bias, col_slice):
        wp = np.zeros((KK * P, 256), f32)
        wp[:DMODEL] = W[:, col_slice]
        wp[DMODEL] = bias[col_slice]
        return wp

    ms = slice(128 * g, 128 * (g + 1))
    wm1p = np.zeros((KK * P, 128), f32)
    wm1p[:DMODEL] = Wm1[:, ms]
    wm1p[DMODEL] = bm1[ms]

    return {
        "xT": ptile(xpad, T),
        "wq": ptile(wpad(Wq, bq, cs), 256),
        "wk": ptile(wpad(Wk, bk, cs), 256),
        "wv": ptile(wpad(Wv, bv, cs), 256),
        "wo": ptile(np.ascontiguousarray(Wo[cs], f32), DMODEL),
        "wm1": ptile(wm1p, 128),
        "wm2": np.ascontiguousarray(Wm2[ms], f32).reshape(P, 1),
        "bm2b": None,  # filled by caller
    }


def kernel(x, Wq, bq, Wk, bk, Wv, bv, Wo, bo, Wm1, bm1, Wm2, bm2):
    global _COMPILED, LAST_RESULTS
    from concourse import bass_utils

    x = np.asarray(x, np.float32)
    B = x.shape[0]
    assert x.shape == (2, T, DMODEL)

    if _COMPILED is None:
        _COMPILED = _build_program()
    nc = _COMPILED

    bm2bc = np.full((P, 1), np.float32(np.asarray(bm2).reshape(-1)[0]))
    in_maps = []
    for c in range(NCORES):
        m = _prep_core_inputs(
            c,
            x,
            np.asarray(Wq, np.float32), np.asarray(bq, np.float32),
            np.asarray(Wk, np.float32), np.asarray(bk, np.float32),
            np.asarray(Wv, np.float32), np.asarray(bv, np.float32),
            np.asarray(Wm1, np.float32), np.asarray(bm1, np.float32),
            np.asarray(Wm2, np.float32), np.asarray(Wo, np.float32),
        )
        m["bm2b"] = bm2bc
        in_maps.append(m)

    kw = {}
    if TRACE:
        kw = dict(trace=True, trace_cores=list(range(NCORES)))
    res = bass_utils.run_bass_kernel_spmd(
        nc, in_maps, core_ids=list(range(NCORES)), **kw
    )
    LAST_RESULTS = res

    bo = np.asarray(bo, np.float32)
    output = np.zeros((B, T, DMODEL), np.float32)
    strengths = np.zeros((B, T), np.float32)
    for b in range(B):
        acc = np.zeros((T, DMODEL), np.float32)
        for g in range(4):
            op = res.results[4 * b + g]["outp"]  # [P, NT, DMODEL]
            acc += op.transpose(1, 0, 2).reshape(T, DMODEL)
        output[b] = acc + bo
        strengths[b] = res.results[4 * b]["souts"].transpose(1, 0).reshape(T)
    return output, strengths
